# revision 40
# baseline (speedup 1.0000x reference)
"""BitLinear (RMSNorm + absmean ternary weight quant + int8 activation quant
+ matmul + dequant) on 8 Trainium2 NeuronCores.

Sharding: data-parallel over the batch dim. Each core gets 2048 tokens
(one batch element) and the full weight; weight quantization is replicated
on every core (no collectives needed).

Math notes:
 - x_q in [-127,127] integers and w_ternary in {-1,0,+1} are both exactly
   representable in bf16; products and fp32 PSUM sums (< 2^24) are exact,
   so the quantized matmul runs on the bf16 tensor engine path bit-exactly.
 - round-half-even is implemented with the +1.5*2^23 magic-number trick.
 - clip(round(w/g'),-1,1) == (w >= 0.5 g') - (w <= -0.5 g'), computed as
   two DVE compare passes producing -w_ternary; the sign is folded into
   the dequant scale (-gamma).
"""

import sys

for _p in ("/opt/trn_rl_repo", "/opt/pypackages"):
    if _p not in sys.path:
        sys.path.append(_p)

import numpy as np

import concourse.bass as bass
import concourse.bacc as bacc
import concourse.tile as tile
from concourse import mybir
from concourse.bass_utils import run_bass_kernel_spmd

P = 128
MAGIC = 12582912.0  # 1.5 * 2^23 : fp32 round-to-nearest-even shifter
EPS = 1e-8
QB = 127.0
F32 = mybir.dt.float32
BF16 = mybir.dt.bfloat16
AF = mybir.ActivationFunctionType
OP = mybir.AluOpType
NFREE = 512  # matmul moving free dim / PSUM bank


def _bcast_row(ap_1d, parts):
    """Broadcast a 1-D AP across `parts` partitions via a 0-stride dim."""
    return bass.AP(
        tensor=ap_1d.tensor, offset=ap_1d.offset, ap=[[0, parts]] + list(ap_1d.ap)
    )


def _recip_newton(nc, pool, name, din):
    """r = 1/din with one Newton refinement. din: [P, c] fp32 AP."""
    shape = [din.shape[0], din.shape[-1]]
    r0 = pool.tile(shape, F32, name=f"{name}_r0")
    nc.vector.reciprocal(out=r0, in_=din)
    t = pool.tile(shape, F32, name=f"{name}_t")
    nc.vector.tensor_mul(t, din, r0)
    nc.vector.tensor_scalar(
        out=t, in0=t, scalar1=-1.0, scalar2=2.0, op0=OP.mult, op1=OP.add
    )
    r1 = pool.tile(shape, F32, name=f"{name}_r1")
    nc.vector.tensor_mul(r1, r0, t)
    return r1


def build_bitlinear(tc, x_d, w_d, b_d, out_d, gneg_row_d, bog_row_d, T, D, N):
    """Emit the kernel for one core: x[T,D] fp32, w[N,D], b[N] -> out[T,N]."""
    from contextlib import ExitStack

    nc = tc.nc
    KT = D // P  # contraction tiles
    DT = N // P  # dout row tiles
    TT = T // P  # token tiles
    NT = N // NFREE  # matmul free-dim tiles
    GW = DT // NT  # weight tiles per n-tile group
    CH = min(2, TT)  # token tiles per chunk
    NC_ = TT // CH  # chunks

    with ExitStack() as ctx:
        const = ctx.enter_context(tc.tile_pool(name="const", bufs=1))
        wq = ctx.enter_context(tc.tile_pool(name="wq", bufs=5))
        fscr = ctx.enter_context(tc.tile_pool(name="fscr", bufs=2))
        wtn_p = ctx.enter_context(tc.tile_pool(name="wtn_p", bufs=3))
        wtT_p = ctx.enter_context(tc.tile_pool(name="wtT_p", bufs=1))
        xin = ctx.enter_context(tc.tile_pool(name="xin", bufs=3))
        xq_p = ctx.enter_context(tc.tile_pool(name="xq_p", bufs=3))
        xqT_p = ctx.enter_context(tc.tile_pool(name="xqT_p", bufs=3))
        ost = ctx.enter_context(tc.tile_pool(name="ost", bufs=3))
        stat = ctx.enter_context(tc.tile_pool(name="stat", bufs=3))
        psum = ctx.enter_context(tc.tile_pool(name="psum", bufs=8, space="PSUM"))

        # ---------------- constants ----------------
        eps_c = const.tile([P, 1], F32)
        nc.vector.memset(eps_c, 1e-8)
        negmagic_c = const.tile([P, 1], F32)
        nc.vector.memset(negmagic_c, -MAGIC)
        zero_c = const.tile([P, 1], F32)
        nc.vector.memset(zero_c, 0.0)
        magic_c = const.tile([P, 1], F32)
        nc.vector.memset(magic_c, MAGIC)

        gssw = const.tile([P, DT], F32)  # sum(|w|) per dout row
        gneg = const.tile([P, DT], F32)  # -gamma = -mean(|w|)
        phalf_g = const.tile([P, DT], F32)
        nhalf_g = const.tile([P, DT], F32)
        # -w_ternary, transposed, one tile per matmul n-tile so the first
        # matmuls only depend on the first GW weight row-tiles:
        # wtTn[n][:, k, f] = -w_t[n*512+f, k*128+p]
        wtTn = [
            wtT_p.tile([P, KT, NFREE], BF16, name=f"wtTn{n}") for n in range(NT)
        ]

        def emit_w_tile(d):
            w_tile = wq.tile([P, D], F32, name="w_tile")
            nc.gpsimd.dma_start(out=w_tile, in_=w_d[d * P : (d + 1) * P, :])
            wabs = fscr.tile([P, D], BF16, name="wabs", tag="wscr", bufs=2)
            nc.scalar.activation(
                out=wabs,
                in_=w_tile,
                func=AF.Abs,
                bias=zero_c[:, :],
                accum_out=gssw[:, d : d + 1],
            )
            ds_ = slice(d, d + 1)
            nc.vector.tensor_scalar(
                out=phalf_g[:, ds_], in0=gssw[:, ds_], scalar1=0.5 / D,
                scalar2=0.5 * EPS, op0=OP.mult, op1=OP.add,
            )
            nc.vector.tensor_scalar(
                out=nhalf_g[:, ds_], in0=gssw[:, ds_], scalar1=-0.5 / D,
                scalar2=-0.5 * EPS, op0=OP.mult, op1=OP.add,
            )
            nc.vector.tensor_scalar(
                out=gneg[:, ds_], in0=gssw[:, ds_], scalar1=-1.0 / D,
                scalar2=None, op0=OP.mult,
            )
            ac = fscr.tile([P, D], BF16, name="ac", tag="wscr2", bufs=2)
            nc.vector.tensor_scalar(
                out=ac, in0=w_tile, scalar1=phalf_g[:, ds_],
                scalar2=None, op0=OP.is_ge,
            )
            wtn = wtn_p.tile([P, D], BF16, name="wtn")  # -w_ternary
            nc.vector.scalar_tensor_tensor(
                out=wtn,
                in0=w_tile,
                scalar=nhalf_g[:, ds_],
                in1=ac,
                op0=OP.is_le,
                op1=OP.subtract,
            )
            # batched block-transpose into this d's n-group tile
            nc.scalar.dma_start_transpose(
                out=wtTn[d // GW][:, :, (d % GW) * P : (d % GW + 1) * P],
                in_=wtn[:, :],
            )

        def emit_xquant_chunk(c):
            """Quantize CH token tiles; returns (xqT tiles, xs chunk tile)."""
            xqTs = []
            x_tiles = []
            ssc = stat.tile([P, CH], F32, name="ssc")
            mc = stat.tile([P, CH], F32, name="mc")
            for jj in range(CH):
                j = c * CH + jj
                x_tile = xin.tile([P, D], F32, name="x_tile")
                nc.sync.dma_start(out=x_tile, in_=x_d[j * P : (j + 1) * P, :])
                sqscr = xq_p.tile([P, D], BF16, name="sqscr", tag="xq")
                nc.scalar.activation(
                    out=sqscr,
                    in_=x_tile,
                    func=AF.Square,
                    bias=zero_c[:, :],
                    accum_out=ssc[:, jj : jj + 1],
                )
                nc.vector.tensor_reduce(
                    out=mc[:, jj : jj + 1],
                    in_=x_tile,
                    axis=mybir.AxisListType.X,
                    op=OP.max,
                    apply_absolute_value=True,
                )
                x_tiles.append(x_tile)
            rmsc = stat.tile([P, CH], F32, name="rmsc")
            nc.scalar.activation(
                out=rmsc, in_=ssc, func=AF.Sqrt, scale=1.0 / D, bias=eps_c[:, :]
            )
            rrmsc = stat.tile([P, CH], F32, name="rrmsc")
            nc.vector.reciprocal(out=rrmsc, in_=rmsc)
            # xs = (m * rrms) / QB  (per-token dequant scale)
            xsc = stat.tile([P, CH], F32, name="xsc")
            nc.vector.tensor_mul(xsc, mc, rrmsc)
            nc.vector.tensor_scalar(
                out=xsc, in0=xsc, scalar1=1.0 / QB, scalar2=None, op0=OP.mult
            )
            # alpha = 1 / (m/QB + rms*1e-8)
            adenc = stat.tile([P, CH], F32, name="adenc")
            nc.vector.tensor_scalar(
                out=adenc, in0=rmsc, scalar1=1e-8, scalar2=None, op0=OP.mult
            )
            nc.vector.scalar_tensor_tensor(
                out=adenc, in0=mc, scalar=1.0 / QB, in1=adenc,
                op0=OP.mult, op1=OP.add,
            )
            alphac = stat.tile([P, CH], F32, name="alphac")
            nc.vector.reciprocal(out=alphac, in_=adenc)
            for jj in range(CH):
                x_tile = x_tiles[jj]
                # x_q = round_half_even(x * alpha), exact in bf16; both
                # affine steps of the magic-number round on ACT, the first
                # in place over x_tile (its last consumer).
                nc.scalar.activation(
                    out=x_tile,
                    in_=x_tile,
                    func=AF.Identity,
                    scale=alphac[:, jj : jj + 1],
                    bias=magic_c[:, :],
                )
                xq = xq_p.tile([P, D], BF16, name="xq", tag="xq")
                nc.scalar.activation(
                    out=xq, in_=x_tile, func=AF.Identity, bias=negmagic_c[:, :]
                )
                xqT = xqT_p.tile([P, KT, P], BF16, name="xqT")
                nc.sync.dma_start_transpose(out=xqT[:, :, :], in_=xq[:, :])
                xqTs.append(xqT)
            return xqTs, xsc

        # ---- interleaved emission: weight tiles + token-chunk quant ----
        chunk_data = {}
        emit_at = {((cc + 1) * DT) // NC_ - 1: cc for cc in range(NC_)}
        for d in range(DT):
            emit_w_tile(d)
            if d in emit_at:
                cc = emit_at[d]
                chunk_data[cc] = emit_xquant_chunk(cc)

        # ------------- broadcast -gamma and bias/(-gamma) rows -------------
        # bog computed in the tiny [P, DT] column domain, then broadcast.
        bias_cols = const.tile([P, DT], F32)
        nc.sync.dma_start(
            out=bias_cols,
            in_=bass.AP(tensor=b_d.tensor, offset=b_d.offset, ap=[[1, P], [P, DT]]),
        )
        grec = _recip_newton(nc, stat, "gr", gneg)
        bog_cols = const.tile([P, DT], F32)
        nc.vector.tensor_mul(bog_cols, bias_cols, grec)
        for d in range(DT):
            nc.scalar.dma_start(
                out=gneg_row_d[d * P : (d + 1) * P], in_=gneg[:, d : d + 1]
            )
            nc.scalar.dma_start(
                out=bog_row_d[d * P : (d + 1) * P], in_=bog_cols[:, d : d + 1]
            )
        gnegB = const.tile([P, N], F32)
        nc.gpsimd.dma_start(out=gnegB, in_=_bcast_row(gneg_row_d, P))
        bogB = const.tile([P, N], F32)
        nc.gpsimd.dma_start(out=bogB, in_=_bcast_row(bog_row_d, P))

        # --------------------------- matmul + dequant ---------------------------
        for c in range(NC_):
            xqTs, xsc = chunk_data[c]
            for n in range(NT):
                ns = slice(n * NFREE, (n + 1) * NFREE)
                for jj in range(CH):
                    j = c * CH + jj
                    ps = psum.tile([P, NFREE], F32, name="ps")
                    for k in range(KT):
                        nc.tensor.matmul(
                            ps[:, :],
                            lhsT=xqTs[jj][:, k, :],
                            rhs=wtTn[n][:, k, :],
                            start=(k == 0),
                            stop=(k == KT - 1),
                        )
                    # out = (psum_neg * xs + bias/(-gamma)) * (-gamma)
                    u = ost.tile([P, NFREE], F32, name="u")
                    nc.vector.scalar_tensor_tensor(
                        out=u,
                        in0=ps,
                        scalar=xsc[:, jj : jj + 1],
                        in1=bogB[:, ns],
                        op0=OP.mult,
                        op1=OP.add,
                    )
                    nc.vector.tensor_mul(u, u, gnegB[:, ns])
                    nc.gpsimd.dma_start(
                        out=out_d[j * P : (j + 1) * P, ns], in_=u
                    )


def build_nc(T, D, N, num_cores=8):
    nc = bacc.Bacc(
        "TRN2", target_bir_lowering=False, debug=False, num_devices=num_cores
    )
    x_d = nc.dram_tensor("x", [T, D], F32, kind="ExternalInput")
    w_d = nc.dram_tensor("weight", [N, D], F32, kind="ExternalInput")
    b_d = nc.dram_tensor("bias", [N], F32, kind="ExternalInput")
    out_d = nc.dram_tensor("out", [T, N], F32, kind="ExternalOutput")
    gneg_row_d = nc.dram_tensor("gneg_row", [N], F32)
    bog_row_d = nc.dram_tensor("bog_row", [N], F32)
    with tile.TileContext(nc) as tc:
        build_bitlinear(
            tc,
            x_d.ap(),
            w_d.ap(),
            b_d.ap(),
            out_d.ap(),
            gneg_row_d.ap(),
            bog_row_d.ap(),
            T,
            D,
            N,
        )
    nc.compile()
    return nc


_CACHE: dict = {}


def get_compiled(T=2048, D=2048, N=2048, num_cores=8):
    key = (T, D, N, num_cores)
    if key not in _CACHE:
        _CACHE[key] = build_nc(T, D, N, num_cores)
    return _CACHE[key]


def run(x, weight, bias, trace=False, **spmd_kwargs):
    x = np.ascontiguousarray(x, dtype=np.float32)
    weight = np.ascontiguousarray(weight, dtype=np.float32)
    bias = np.ascontiguousarray(bias, dtype=np.float32)
    B, S, D = x.shape
    N = weight.shape[0]
    num_cores = 8
    T = (B * S) // num_cores
    nc = get_compiled(T, D, N, num_cores)
    xs = x.reshape(num_cores, T, D)
    in_maps = [
        {"x": xs[c], "weight": weight, "bias": bias} for c in range(num_cores)
    ]
    res = run_bass_kernel_spmd(
        nc, in_maps, list(range(num_cores)), trace=trace, **spmd_kwargs
    )
    out = np.stack([res.results[c]["out"] for c in range(num_cores)])
    return out.reshape(B, S, N).astype(np.float32), res


def kernel(x, weight, bias):
    out, _ = run(x, weight, bias)
    return out


if __name__ == "__main__":
    rng = np.random.default_rng(0)
    x = rng.standard_normal((8, 2048, 2048), dtype=np.float32)
    w = rng.uniform(-0.05, 0.05, (2048, 2048)).astype(np.float32)
    b = (rng.standard_normal(2048) * 0.02).astype(np.float32)
    out = kernel(x, w, b)
    print(out.shape, out.dtype)
